# revision 27
# baseline (speedup 1.0000x reference)
"""Trainium2 Bass kernel for nn_NoBrainEncoderBlock_31662498906140.

out = softmax_n( clip( cos(q1_row, k1_row_n) * mask, 0, 1 ) )

Only q1, k1, mask affect the output (q2, k2, temp are unused by the math),
so only those are transferred to the device.

Sharding: data-parallel over batch B=32 across 8 NeuronCores (BPC=4 rows
per core); all reductions are per-row over D or per-row softmax over N, so
there is no cross-core communication.

Per-core layout (natural, no transpose): each k1 tile is [128 part, D]
where partition p holds k1[b, p*NT + t, :] (NT = N/128 = 16, t = tile id).
Two fused single-pass reductions per tile:
  - dot[n]  = sum_d k1[n,d] * q1n[d]  via DVE scalar_tensor_tensor(accum_out)
  - sq[n]   = sum_d k1[n,d]^2         via ACT activation(Square, accum_out)
Measured per-core chains: DMA 156 us (431 GB/s), DVE 66-op chain ~185 us,
ACT 62-op chain ~183 us — compute-bound, balanced by shifting two of the
square reductions from ACT to DVE.

q1 is L2-normalized on the host (tiny [32,2048]); the per-n cosine is then
dot * rsqrt(sq), multiplied by mask, clipped, exp'ed (scores in [0,1] so no
max subtraction needed) and normalized by the per-row sum (partition
all-reduce of the per-partition exp sums).

rsqrt is computed as exp(-0.5*ln(x)) so every ACT function used (square,
ln, exp) lives in the single `natural_log_exp_and_others` table set — no
table reloads in steady state. q1 rows are broadcast across partitions
on-chip via gpsimd.partition_broadcast (GPSIMD is otherwise idle), keeping
broadcast traffic off HBM. 8x 2 MiB-deep DMA double buffering; per-row
softmax tails interleave with the next row's stream.
"""

import numpy as np

B, N, D = 32, 2048, 2048
NCORES = 8
BPC = B // NCORES      # 4 batch rows per core
P = 128                # SBUF partitions
NT = N // P            # 16 n-columns per partition
BD = BPC * NT          # 64 stat columns per core
KBUFS = 8              # k1 tile buffer depth (8 x 2 MiB SBUF)
VPAIR = 2              # n-rows per partition per DMA (2 MiB transfers)

_FNS = {}              # reps -> (jitted_fn, mesh, sharding)


def _build_tile_program(nc, tile, mybir, bass_isa, q1n, k1, mask, bpc, n, d, reps=1):
    """Emit the per-core Tile program. q1n [bpc,d], k1 [bpc,n,d], mask [bpc,n].

    reps > 1 wraps the whole body in an on-device loop executing the exact
    same computation `reps` times — used only for timing (amortizes host
    dispatch latency out of the measurement)."""
    from contextlib import ExitStack, nullcontext

    f32 = mybir.dt.float32
    p = 128
    nt = n // p
    bd = bpc * nt

    v = VPAIR
    nu = nt // v
    out = nc.dram_tensor("out", [bpc, n], f32, kind="ExternalOutput")

    # tile u: partition p holds rows n = p*nt + u*v .. p*nt + u*v + v-1,
    # i.e. v consecutive rows = v*d contiguous f32 per partition per DMA
    k1r = k1[:, :, :].rearrange("b (p u v) d -> b u p (v d)", p=p, u=nu, v=v)
    mask_r = mask[:, :].rearrange("b (p j) -> p b j", p=p)        # [128, bpc, nt]
    out_r = out[:, :].rearrange("b (p j) -> p b j", p=p)          # [128, bpc, nt]

    with tile.TileContext(nc) as tc, ExitStack() as ctx:
        stat = ctx.enter_context(tc.tile_pool(name="stat", bufs=1))
        kpool = ctx.enter_context(tc.tile_pool(name="kp", bufs=KBUFS))
        scr = ctx.enter_context(tc.tile_pool(name="scr", bufs=4))
        sqscr = ctx.enter_context(tc.tile_pool(name="sqscr", bufs=4))

        ctx.enter_context(tc.For_i(0, reps, 1) if reps > 1 else nullcontext())

        # q1n rows -> one partition, then broadcast across all 128 partitions
        # on GPSIMD (otherwise idle) so the 4 MiB of broadcast stays off HBM.
        qsmall = stat.tile([1, bpc * d], f32, tag="qsmall")
        nc.sync.dma_start(
            out=qsmall[:].rearrange("p (b d) -> p b d", b=bpc), in_=q1n[:, :]
        )
        qb = []
        for b in range(bpc):
            t = stat.tile([p, d], f32, tag=f"q{b}")
            nc.gpsimd.partition_broadcast(t, qsmall[:, b * d : (b + 1) * d])
            qb.append(t)

        mask_all = stat.tile([p, bd], f32, tag="mask")
        nc.sync.dma_start(
            out=mask_all[:].rearrange("p (b j) -> p b j", b=bpc), in_=mask_r
        )

        dot_all = stat.tile([p, bd], f32, tag="dot")
        sq_all = stat.tile([p, bd], f32, tag="sq")
        # per-row tail tiles (tails interleave with the next row's DMAs)
        rsq = stat.tile([p, bd], f32, tag="rsq")
        rsq2 = stat.tile([p, bd], f32, tag="rsq2")
        cosm = stat.tile([p, bd], f32, tag="cosm")
        s = stat.tile([p, bd], f32, tag="s")
        sc = stat.tile([p, bd], f32, tag="sc")
        e_all = stat.tile([p, bd], f32, tag="e")
        esum = stat.tile([p, bpc], f32, tag="esum")
        tot = stat.tile([p, bpc], f32, tag="tot")
        rtot = stat.tile([p, bpc], f32, tag="rtot")
        o_all = stat.tile([p, bd], f32, tag="o")

        for b in range(bpc):
            for u in range(nu):
                kt = kpool.tile([p, v * d], f32, tag="kt")
                nc.sync.dma_start(out=kt, in_=k1r[b, u])
                for vv in range(v):
                    col = b * nt + u * v + vv
                    kslice = kt[:, vv * d : (vv + 1) * d]
                    dot_out = scr.tile([p, 1], f32, tag="dotout")
                    nc.vector.scalar_tensor_tensor(
                        out=dot_out.broadcast_to([p, d]),
                        in0=kslice,
                        scalar=1.0,
                        in1=qb[b],
                        op0=mybir.AluOpType.mult,
                        op1=mybir.AluOpType.mult,
                        accum_out=dot_all[:, col : col + 1],
                    )
                    if col in (16, 17):
                        # ACT's 64-op square chain (~189us) runs ~5% behind
                        # DVE's dot chain (~179us); shift two squares to DVE
                        sq_out = sqscr.tile([p, 1], f32, tag="sqoutv")
                        nc.vector.scalar_tensor_tensor(
                            out=sq_out.broadcast_to([p, d]),
                            in0=kslice,
                            scalar=1.0,
                            in1=kslice,
                            op0=mybir.AluOpType.mult,
                            op1=mybir.AluOpType.mult,
                            accum_out=sq_all[:, col : col + 1],
                        )
                    else:
                        sq_out = sqscr.tile([p, 1], f32, tag="sqout")
                        nc.scalar.activation(
                            out=sq_out.broadcast_to([p, d]),
                            in_=kslice,
                            func=mybir.ActivationFunctionType.Square,
                            accum_out=sq_all[:, col : col + 1],
                        )

            # ---- row tail: cos -> mask -> clip -> softmax ----
            # rsqrt(x) = exp(-0.5*ln(x)); ln/exp/square share one table set
            sl = slice(b * nt, (b + 1) * nt)
            nc.scalar.activation(
                out=rsq[:, sl], in_=sq_all[:, sl],
                func=mybir.ActivationFunctionType.Ln,
            )
            nc.scalar.activation(
                out=rsq2[:, sl], in_=rsq[:, sl],
                func=mybir.ActivationFunctionType.Exp, scale=-0.5,
            )
            nc.vector.tensor_mul(cosm[:, sl], dot_all[:, sl], rsq2[:, sl])
            nc.vector.tensor_mul(s[:, sl], cosm[:, sl], mask_all[:, sl])
            nc.vector.tensor_scalar(
                out=sc[:, sl], in0=s[:, sl], scalar1=0.0, scalar2=1.0,
                op0=mybir.AluOpType.max, op1=mybir.AluOpType.min,
            )
            nc.scalar.activation(
                out=e_all[:, sl], in_=sc[:, sl],
                func=mybir.ActivationFunctionType.Exp,
                accum_out=esum[:, b : b + 1],
            )
            nc.gpsimd.partition_all_reduce(
                tot[:, b : b + 1], esum[:, b : b + 1], p, bass_isa.ReduceOp.add
            )
            nc.vector.reciprocal(rtot[:, b : b + 1], tot[:, b : b + 1])
            nc.vector.tensor_scalar_mul(
                o_all[:, sl], e_all[:, sl], rtot[:, b : b + 1]
            )
            nc.sync.dma_start(out=out_r[:, b, :], in_=o_all[:, sl])

    return out


def _get_fn(reps=1):
    if reps in _FNS:
        return _FNS[reps]
    import jax
    from jax.sharding import Mesh, NamedSharding, PartitionSpec
    import concourse.tile as tile
    from concourse import bass_isa, mybir
    from concourse.bass2jax import bass_jit, bass_shard_map

    @bass_jit
    def shard_kernel(nc, q1n, k1, mask):
        return _build_tile_program(
            nc, tile, mybir, bass_isa, q1n, k1, mask, BPC, N, D, reps=reps
        )

    devices = jax.devices()[:NCORES]
    mesh = Mesh(np.array(devices), ("core",))
    pspec = PartitionSpec("core")
    fn = bass_shard_map(
        shard_kernel,
        mesh=mesh,
        in_specs=(pspec, pspec, pspec),
        out_specs=pspec,
    )
    _FNS[reps] = (fn, mesh, NamedSharding(mesh, pspec))
    return _FNS[reps]


def _prep_inputs(q1, k1, mask):
    q1 = np.ascontiguousarray(np.asarray(q1, dtype=np.float32))
    k1 = np.ascontiguousarray(np.asarray(k1, dtype=np.float32))
    mask = np.ascontiguousarray(np.asarray(mask, dtype=np.float32))
    nrm = np.linalg.norm(q1, axis=-1, keepdims=True)
    q1n = q1 / np.maximum(nrm, 1e-12)
    return q1n, k1, mask


def kernel(q1, k1, q2, k2, mask, temp):
    q1n, k1, mask = _prep_inputs(q1, k1, mask)
    fn, _, _ = _get_fn()
    out = fn(q1n, k1, mask)
    return np.asarray(out, dtype=np.float32).reshape(B, N)
